# revision 36
# baseline (speedup 1.0000x reference)
"""Trainium2 Bass kernel for the 8x8-block rfft2 magnitude ("DCT") layer.

Computes, for input x [32,1,512,512] f32 and freq_weights [64] f32:
  per 8x8 spatial block: |rfft2(block, norm='ortho')| -> 40 freq bins,
  scaled by sigmoid(freq_weights)[:40], zero-padded to 64 channels.
Output: [32, 64, 64, 64] f32.

Pure data parallel: 4 images per core on 8 cores. Per core, 8 slabs of
256 rows ([128, 1024] paired loads, one DMA each). The separable 8-pt
DFT runs as two matmul stages with a PSUM->SBUF bf16 copy between:

  stage 1 (per 128-col chunk q, per row-half h): data stationary
    (fp32r), W1 moving (fp32r, N=256; fp32r needs N>=256 for
    1 col/cycle): vertical DFT of every row-block, transposed so
    spatial columns (bjl,j) land on partitions.
  stage 2 (per chunk, per row-parity l): Z stationary (bf16 contiguous
    slices), [C|S] / [-S|C] bf16 moving (N=160): accumulating matmul
    pair gives Re/Im of the 2D DFT at psum partitions p=(t,u).

  Elementwise tail per slab: 2 Z-copies [128,1024] PSUM->SBUF bf16
    (DVE - the serial chain that sets the steady-state cadence,
    ~1.2us each; HW allows only ONE PSUM input per instruction and
    GPSIMD has no PSUM port, so the drain work can only go to
    DVE/ACT), 2 squares [128,640] PSUM->SBUF bf16 (ACT), 4 bf16 adds
    (GPSIMD/Pool), 1 sqrt [128,640] bf16 (ACT) with sigmoid(w) folded
    into the activation scale when freq_weights is uniform.

  Store: one contiguous [128, 640] bf16 tile per slab (p=(t,u),
    f=(v,l,q)) into a raw [8,128,640] bf16 output on the ACT HWDGE
    ring (st=scalar, ~-2us vs sharing the SP ring with loads); the
    host reorders/casts to [B,64,64,64] f32 and fills ch 40..63 with
    zeros. Halves store traffic vs f32 and replaces the old scattered
    512B-run store with 1280B-contiguous runs.

  Emission is software-pipelined (front k / mid k-4 / tail k-7). The
  repeat-loop timing path unrolls the body (unroll_rep) because each
  For_i staggered-reset boundary costs ~9us of pipeline drain; ur=16
  amortizes it to ~0.5us.

  HW-measured (repeat-loop slope, 8 cores concurrent, full problem per
  iteration): ~21.85us/core vs 33.1us for the session-start baseline
  (f32 store) and ~50.4us for v1. The plateau is real: DVE copy chain
  19.4us + ~0.3us/slab sem/handoff latency. Also tested and rejected:
  mid-before-front emission (mfirst - PE queue reorder, +0.8us),
  per-slab copy migration to ACT (queue coupling), DMA-assisted PSUM
  drain (fabric budget + no dma-cast), ur=32 (icache pressure). Steady-state floor is the 16-copy
  DVE chain (~19.4us) + sem slack; DMA is ~2.1us/slab and fully
  hidden. Failed experiments (kept as cfg options, off by default):
  c8=1 compact-Z via conjugate symmetry - the model loves it but HW
  runs multi-dim strided/short-run engine copies 3-4x slower than the
  1 el/cycle model, 74us total; per-slab copy-engine alternation
  (cps) similarly loses to queue head-of-line coupling.
"""

import math
import numpy as np
from contextlib import ExitStack

import ml_dtypes
import concourse.bacc as bacc
import concourse.mybir as mybir
from concourse import tile
from concourse.bass_utils import run_bass_kernel_spmd

F32 = mybir.dt.float32
F32R = mybir.dt.float32r
BF16 = mybir.dt.bfloat16

N_CORES = 8
IMGS_PER_CORE = 4
SLABS_PER_IMG = 2  # 512 rows / 256


def _build_host_matrices_v2(freq_weights: np.ndarray, c8: bool = False):
    p = np.arange(128)
    bi_r, i_r = p // 8, p % 8
    # W1 [128, 256]: n = (l2, reim2, thl8, u8) -- or (l2, reim2, u8, thl8)
    # for the c8 layout (u-outer m-blocks keep the compact-copy runs
    # contiguous); bil = 2*thl + l; DFT /8
    n = np.arange(256)
    l_n = n // 128
    reim_n = (n % 128) // 64
    if c8:
        u_n = (n % 64) // 8
        th_n = n % 8
    else:
        th_n = (n % 64) // 8
        u_n = n % 8
    bil_n = 2 * th_n + l_n
    ang = 2 * math.pi * np.outer(i_r, u_n) / 8.0
    W1 = np.where(reim_n[None, :] == 0, np.cos(ang), np.sin(ang)) / 8.0
    W1 *= bi_r[:, None] == bil_n[None, :]

    # C2/S2 [128 p=(bjl,j), 80 m=(v,bjl2)]
    bjl_p, j_p = p // 8, p % 8
    m = np.arange(80)
    v_m, bjl2_m = m // 16, m % 16
    blk = bjl_p[:, None] == bjl2_m[None, :]
    ang2 = 2 * math.pi * v_m[None, :] * j_p[:, None] / 8.0
    C2 = np.cos(ang2) * blk
    S2 = np.sin(ang2) * blk
    CS = np.concatenate([C2, S2], axis=1)
    SNC = np.concatenate([-S2, C2], axis=1)
    cstw = W1.astype(np.float32)  # [128, 256] fp32r stage-1 stream
    cstb = np.concatenate([CS, SNC], axis=1)  # [128, 320]
    cstb = cstb.astype(np.float32).astype(ml_dtypes.bfloat16)

    # Wtile [128 p, 640 f=(v,l,q)] -> sigmoid(fw)[u*5+v]
    # p = (u8,t16) for c8 layout, else (t16,u8)
    w = 1.0 / (1.0 + np.exp(-freq_weights.astype(np.float64)))
    u_idx = np.arange(128) // 16 if c8 else np.arange(128) % 8
    v_idx = np.arange(640) // 128
    Wtile = w[u_idx[:, None] * 5 + v_idx[None, :]].astype(ml_dtypes.bfloat16)
    return cstw, cstb, Wtile


_NC_CACHE = {}


def _build_bass_v2(n_imgs: int = IMGS_PER_CORE, repeat: int = 1, cfg: dict = None):
    cfg = dict(cfg or {})
    uni_w = cfg.get("uniform_w")
    cp_eng = cfg.get("cp", "vv")    # z-copy per super: v=DVE, a=ACT
    sq_eng = cfg.get("sq", "aa")    # square per po super: a=ACT, v=DVE
    add_eng = cfg.get("add", "gggg")  # add per po: g=GPSIMD(Pool), v=DVE
    wm_eng = cfg.get("wm", "g")
    ld_ring = cfg.get("ld", "sync")
    st_ring = cfg.get("st", "scalar")
    ab = cfg.get("a", 6)
    zb = cfg.get("z", 10)     # [128,1024] bf16 super tiles; 2 per slab
    sqb = cfg.get("sqb", 10)   # [128, 640]; 2 per slab
    ssb = cfg.get("ss", 7)
    rtb = cfg.get("rt", 7)
    pszb = cfg.get("psz", 2)  # 2-bank super-tiles
    psob = cfg.get("pso", 2)  # 2-bank super-tiles

    nc = bacc.Bacc("TRN2", target_bir_lowering=False)
    x = nc.dram_tensor("x", [n_imgs * 512, 512], F32R, kind="ExternalInput")
    cstw = nc.dram_tensor("cstw", [128, 256], F32R, kind="ExternalInput")
    cstb = nc.dram_tensor("cstb", [128, 320], BF16, kind="ExternalInput")
    if uni_w is None:
        wt = nc.dram_tensor("wt", [128, 640], BF16, kind="ExternalInput")
    # raw bf16 layout: one contiguous [128, 640] tile per slab
    # (p=(t,u), f=(v,l,q)); host reorders to [B,64,64,64] f32. Halves
    # store traffic vs f32 and stores in 1280B-contig runs per partition
    # instead of 512B scattered runs.
    out = nc.dram_tensor(
        "out", [n_imgs * SLABS_PER_IMG, 128, 640], BF16, kind="ExternalOutput"
    )

    ld = getattr(nc, ld_ring)
    st = getattr(nc, st_ring)

    with tile.TileContext(nc) as tc, ExitStack() as ctx:
        consts = ctx.enter_context(tc.tile_pool(name="consts", bufs=1))
        a_pool = ctx.enter_context(tc.tile_pool(name="a", bufs=ab))
        z_pool = ctx.enter_context(tc.tile_pool(name="z", bufs=zb))
        sq_pool = ctx.enter_context(tc.tile_pool(name="sq", bufs=sqb))
        ss_pool = ctx.enter_context(tc.tile_pool(name="ss", bufs=ssb))
        rt_pool = ctx.enter_context(tc.tile_pool(name="rt", bufs=rtb))
        psz_pool = ctx.enter_context(tc.tile_pool(name="psz", bufs=pszb, space="PSUM"))
        pso_pool = ctx.enter_context(tc.tile_pool(name="pso", bufs=psob, space="PSUM"))

        def do_copy(eng, dst, src):
            if eng == "v":
                nc.vector.tensor_copy(dst, src)
            elif eng == "g":
                nc.gpsimd.tensor_copy(dst, src)
            else:
                nc.scalar.copy(dst, src)

        def do_square(eng, dst, src):
            if eng == "a":
                nc.scalar.square(dst, src)
            elif eng == "g":
                nc.gpsimd.tensor_mul(dst, src, src)
            else:
                nc.vector.tensor_mul(dst, src, src)

        def do_add(eng, dst, a, b):
            (nc.gpsimd.tensor_add if eng == "g" else nc.vector.tensor_add)(
                dst, a, b
            )

        def do_dup(eng, dst, src):
            do_copy(eng, dst, src)

        def do_negdup(eng, dst, src):
            if eng == "g":
                nc.gpsimd.tensor_scalar_mul(dst, src, -1.0)
            elif eng == "a":
                nc.scalar.activation(
                    dst, src, mybir.ActivationFunctionType.Copy, 0.0, -1.0
                )
            else:
                nc.vector.tensor_scalar_mul(dst, src, -1.0)

        # pre-issue the first x-slab loads ahead of the const DMAs so the
        # load pipeline starts at t~0 (consts only gate stage-1 matmuls).
        # only valid for repeat==1 (inside a repeat loop slabs reload).
        npre = cfg.get("npre", 0) if repeat == 1 else 0
        pre_a = []
        for pi in range(npre):
            img, s = pi // SLABS_PER_IMG, pi % SLABS_PER_IMG
            a_t = a_pool.tile([128, 1024], F32R)
            row0 = img * 512 + s * 256
            src = x[row0 : row0 + 256, :].rearrange("(h p) c -> p h c", h=2)
            ld.dma_start(a_t[:].rearrange("p (h c) -> p h c", h=2), src)
            pre_a.append(a_t)

        cstw_t = consts.tile([128, 256], F32R, tag="cstw")
        cst_t = consts.tile([128, 320], BF16, tag="cstb")
        nc.sync.dma_start(cstw_t[:], cstw[:])  # W1 first
        nc.sync.dma_start(cst_t[:], cstb[:])
        w1_t = cstw_t[:]
        cs_t = cst_t[:, 0:160]
        snc_t = cst_t[:, 160:320]
        if uni_w is None:
            wt_t = consts.tile([128, 640], BF16, tag="wt")
            nc.sync.dma_start(wt_t[:], wt[:])

        # warm ACT tables (Square, Sqrt) at t=0
        warm = consts.tile([128, 8], F32, tag="warm")
        nc.gpsimd.memset(warm[:], 0.0)
        nc.scalar.square(warm[:], warm[:])
        nc.scalar.sqrt(warm[:], warm[:])
        # warm the PE p-state during the const/first-load DMA window:
        # junk matmuls keep PE continuously busy so the first real
        # stage-1 runs at full clock (cold PE is ~3.7x slower)
        npew = cfg.get("pew", 6)
        if npew:
            warm2 = consts.tile([128, 512], BF16, tag="warm2")
            nc.vector.memset(warm2[:], 0.0)
            pw_t = pso_pool.tile([128, 1024], F32, tag="po")
            for _ in range(npew):
                nc.tensor.matmul(
                    pw_t[:, 0:512], warm2[:, 0:128], warm2[:],
                    start=True, stop=True,
                )

        slabs = [(img, s) for img in range(n_imgs) for s in range(SLABS_PER_IMG)]

        psing = cfg.get("psing", 0)  # 1: psz/z per-chunk 1-bank tiles
        lda = cfg.get("lda")    # per-slab load-ring rotation, e.g. ["sync","scalar"]
        sta = cfg.get("sta")    # per-slab store-ring rotation
        c8 = cfg.get("c8", 0)   # compact-z: copy 10/16 stage-1 comps, mirror rest
        dre_eng = cfg.get("dre", "v")  # reR dup engine
        dim_eng = cfg.get("dim", "g")  # imR' negated-dup engine
        if psing:
            c8 = 0

        def emit_front(img, s, a_pre=None, cp=None):
            """load + stage 1 + Z copies; psz f = (r2,h2,thl8,u8,l2), t=8h+thl.
            psing: per-chunk [128,512] psz/z tiles (fine-grained rotation);
            else 2-bank super-tiles with one copy per chunk-pair."""
            cp = cp or cp_eng
            if a_pre is not None:
                a_t = a_pre
            else:
                a_t = a_pool.tile([128, 1024], F32R)
                row0 = img * 512 + s * 256
                src = x[row0 : row0 + 256, :].rearrange("(h p) c -> p h c", h=2)
                ldq = ld
                if lda:
                    ldq = getattr(nc, lda[(img * SLABS_PER_IMG + s) % len(lda)])
                if not nold:
                    ldq.dma_start(
                        a_t[:].rearrange("p (h c) -> p h c", h=2), src
                    )
            z_ts = []
            if psing:
                for q in range(4):
                    psz_t = psz_pool.tile([128, 512], F32, tag="psz")
                    psz5 = psz_t[:].rearrange(
                        "p (lr h m) -> p lr h m", lr=4, h=2, m=64
                    )
                    for h in range(2):
                        nc.tensor.matmul(
                            psz5[:, :, h, :],
                            a_t[:, 512 * h + 128 * q : 512 * h + 128 * q + 128],
                            w1_t,
                            start=True,
                            stop=True,
                        )
                    z_t = z_pool.tile([128, 512], BF16)
                    do_copy(cp[q], z_t[:], psz_t[:])
                    z_ts.append(z_t[:])
                return z_ts
            for g in range(2):
                psz_t = psz_pool.tile([128, 1024], F32, tag="psz")
                for c in range(2):
                    q = 2 * g + c
                    psz5 = psz_t[:, 512 * c : 512 * c + 512].rearrange(
                        "p (lr h m) -> p lr h m", lr=4, h=2, m=64
                    )
                    for h in range(2):
                        nc.tensor.matmul(
                            psz5[:, :, h, :],
                            a_t[:, 512 * h + 128 * q : 512 * h + 128 * q + 128],
                            w1_t,
                            start=True,
                            stop=True,
                        )
                z_t = z_pool.tile([128, 1024], BF16)
                if c8:
                    # compact copy: only u=0..4 of re/im (conjugate
                    # symmetry makes u=5..7 mirrors), transposing to
                    # u-outer within each (c,l,reim) 128-block:
                    #   z block = [re0..4 | reR(5..7)] / [im0..4 | imR']
                    src_v = psz_t[:].rearrange(
                        "p (m h u t) -> p m u h t", m=8, h=2, u=8, t=8
                    )[:, :, 0:5]
                    dst_v = z_t[:].rearrange(
                        "p (m u h t) -> p m u h t", m=8, u=8, h=2, t=8
                    )[:, :, 0:5]
                    do_copy(cp[g], dst_v, src_v)
                    # mirrors: re(8-u) = re(u); stored-im(8-u) = -im(u)
                    zv = z_t[:].rearrange(
                        "p (cl r u ht) -> p cl r u ht", cl=4, r=2, u=8, ht=16
                    )
                    do_dup(dre_eng, zv[:, :, 0, 5:8, :], zv[:, :, 0, 3:0:-1, :])
                    do_negdup(
                        dim_eng, zv[:, :, 1, 5:8, :], zv[:, :, 1, 3:0:-1, :]
                    )
                else:
                    do_copy(cp[g], z_t[:], psz_t[:])
                z_ts.append(z_t[:, 0:512])
                z_ts.append(z_t[:, 512:1024])
            return z_ts

        osup = cfg.get("osup", 1)  # 1: pso 2-bank supers; 0: 1-bank po tiles

        def emit_mid(z_ts, sq=None, ad=None):
            """stage 2 + squares + adds -> ssum tile. osup: pso super-tile g
            holds po(l=0) at 0:320, po(l=1) at 512:832 for chunk-pair g with
            one square per super; else per-po 1-bank tiles."""
            sq = sq or sq_eng
            ad = ad or add_eng
            ssum_t = ss_pool.tile([128, 640], BF16, tag="ssum")
            ss6 = ssum_t[:].rearrange(
                "p (v l P c b) -> p l P c v b", v=5, l=2, P=2, c=2, b=16
            )
            for pair in range(2):
                if osup:
                    po_t = pso_pool.tile([128, 1024], F32, tag="po")
                    po_parts = [
                        po_t[:, 0:320],
                        po_t[:, 512:832],
                    ]
                else:
                    po_parts = None
                sqs = []
                for l in range(2):
                    if osup:
                        dst_po = po_parts[l]
                    else:
                        po_s = pso_pool.tile([128, 320], F32, tag="po")
                        dst_po = po_s[:]
                    for ci, q in enumerate((2 * pair, 2 * pair + 1)):
                        zq = z_ts[q]
                        dst = dst_po[:, 160 * ci : 160 * ci + 160]
                        nc.tensor.matmul(
                            dst, zq[:, 256 * l : 256 * l + 128],
                            cs_t, start=True, stop=False,
                        )
                        nc.tensor.matmul(
                            dst, zq[:, 256 * l + 128 : 256 * l + 256],
                            snc_t, start=False, stop=True,
                        )
                    if not osup:
                        sq_t = sq_pool.tile([128, 320], BF16, tag="sq")
                        do_square(sq[2 * pair + l], sq_t[:], dst_po)
                        sqs.append(sq_t)
                if osup:
                    # one square over both l-halves of the super
                    sq_t = sq_pool.tile([128, 640], BF16, tag="sq")
                    po_v = po_t[:].rearrange(
                        "p (l m) -> p l m", l=2, m=512
                    )[:, :, 0:320]
                    sq_v = sq_t[:].rearrange("p (l m) -> p l m", l=2, m=320)
                    do_square(sq[pair], sq_v, po_v)
                    sq6 = sq_t[:].rearrange(
                        "p (li c h v b) -> p li h c v b", li=2, c=2, h=2, v=5, b=16
                    )
                    sq_ins = [(sq6[:, l, 0], sq6[:, l, 1]) for l in range(2)]
                else:
                    sq_ins = []
                    for l in range(2):
                        s5 = sqs[l][:].rearrange(
                            "p (c h v b) -> p h c v b", c=2, h=2, v=5, b=16
                        )
                        sq_ins.append((s5[:, 0], s5[:, 1]))
                for l in range(2):
                    do_add(
                        ad[2 * pair + l],
                        ss6[:, l, pair], sq_ins[l][0], sq_ins[l][1],
                    )
            return ssum_t

        def emit_tail(img, s, ssum_t, split=False):
            """sqrt(+weight) + store (bf16 raw tile). split: 2 half-ops
            (halves the last-slab drain latency; +1 ACT op bubble)"""
            root_t = rt_pool.tile([128, 640], BF16, tag="root")
            halves = ((0, 320), (320, 640)) if split else ((0, 640),)
            for lo, hi in halves:
                if uni_w is not None:
                    nc.scalar.activation(
                        root_t[:, lo:hi], ssum_t[:, lo:hi],
                        mybir.ActivationFunctionType.Sqrt,
                        0.0, float(uni_w) * float(uni_w),
                    )
                else:
                    rootf_t = rt_pool.tile([128, 640], F32, tag="rootf")
                    nc.scalar.sqrt(rootf_t[:, lo:hi], ssum_t[:, lo:hi])
                    (nc.gpsimd.tensor_mul if wm_eng == "g"
                     else nc.vector.tensor_mul)(
                        root_t[:, lo:hi], rootf_t[:, lo:hi], wt_t[:, lo:hi]
                    )
                stq = st
                if sta:
                    stq = getattr(nc, sta[(img * SLABS_PER_IMG + s) % len(sta)])
                if not nost:
                    stq.dma_start(
                        out[img * SLABS_PER_IMG + s, :, lo:hi], root_t[:, lo:hi]
                    )

        # software-pipelined emission: each engine's strict-FIFO queue gets
        # work ordered by data availability (front k+1 before mid k before
        # tail k-1), so no stage head-of-line-blocks the next slab's work.
        d_mid = cfg.get("dmid", 4)    # front-to-mid emission lag (slabs)
        d_tail = cfg.get("dtail", 3)  # mid-to-tail lag

        unroll_rep = cfg.get("unroll_rep", 0)
        if unroll_rep:
            slabs = slabs * unroll_rep
        stag = cfg.get("stag", 1)
        if repeat > 1 and cfg.get("pbar", 0):
            tc.prologue_barrier()
        rep_ctx = (
            tc.For_i(0, repeat, 1, staggered_reset=bool(stag))
            if repeat > 1
            else None
        )
        if rep_ctx is not None:
            rep_ctx.__enter__()
        # explicit staggered-reset stage boundaries at slab boundaries
        # (auto-split cuts mid-slab and skews the 4 stage mini-barriers)
        sbnd = cfg.get("sbnd", 0)
        bnd_after = (
            {len(slabs) // 4 - 1, len(slabs) // 2 - 1, 3 * len(slabs) // 4 - 1}
            if (rep_ctx is not None and stag and sbnd)
            else set()
        )
        nold = cfg.get("nold", 0)   # ablation: skip load DMAs
        nost = cfg.get("nost", 0)   # ablation: skip store DMAs
        nomid = cfg.get("nomid", 0)  # ablation: front stage only
        cps = cfg.get("cps")    # optional per-slab cp strings
        sqs = cfg.get("sqs")
        adds = cfg.get("adds")
        rsp = cfg.get("rsp", 0)  # split tail for the last rsp slabs
        nsl = len(slabs)
        tail_i = 0

        def pop_tail(mi, ms, mss):
            nonlocal tail_i
            emit_tail(mi, ms, mss, split=tail_i >= nsl - rsp)
            tail_i += 1

        mfirst = cfg.get("mfirst", 0)  # emit mid/tail before front each
        # round: ready mm2/sq work precedes mm1 that waits on PSUM
        # recycling, avoiding PE-queue head-of-line blocking
        fronts, mids = [], []
        for si, (img, s) in enumerate(slabs):
            a_pre = pre_a[si] if si < npre else None
            cp_s = cps[si % len(cps)] if cps else None
            if mfirst:
                if len(fronts) >= d_mid:
                    fi, fs, fz = fronts.pop(0)
                    fsi = si - d_mid
                    if not nomid:
                        mids.append((fi, fs, emit_mid(
                            fz,
                            sq=sqs[fsi % len(sqs)] if sqs else None,
                            ad=adds[fsi % len(adds)] if adds else None,
                        )))
                if len(mids) > d_tail:
                    mi, ms, mss = mids.pop(0)
                    pop_tail(mi, ms, mss)
                fronts.append(
                    (img, s, emit_front(img, s, a_pre=a_pre, cp=cp_s))
                )
                if si in bnd_after:
                    tc.stage_boundary()
                continue
            fronts.append(
                (img, s, emit_front(img, s, a_pre=a_pre, cp=cp_s))
            )
            if len(fronts) > d_mid:
                fi, fs, fz = fronts.pop(0)
                fsi = si - d_mid
                if not nomid:
                    mids.append((fi, fs, emit_mid(
                        fz,
                        sq=sqs[fsi % len(sqs)] if sqs else None,
                        ad=adds[fsi % len(adds)] if adds else None,
                    )))
            if si in bnd_after:
                tc.stage_boundary()
            if len(mids) > d_tail:
                mi, ms, mss = mids.pop(0)
                pop_tail(mi, ms, mss)
        nxt = len(slabs) - len(fronts)
        while fronts:
            fi, fs, fz = fronts.pop(0)
            if not nomid:
                mids.append((fi, fs, emit_mid(
                    fz,
                    sq=sqs[nxt % len(sqs)] if sqs else None,
                    ad=adds[nxt % len(adds)] if adds else None,
                )))
            nxt += 1
            while len(mids) > d_tail:
                mi, ms, mss = mids.pop(0)
                pop_tail(mi, ms, mss)
        while mids:
            mi, ms, mss = mids.pop(0)
            pop_tail(mi, ms, mss)
        if rep_ctx is not None:
            rep_ctx.__exit__(None, None, None)
    nc.finalize()
    return nc


def kernel(x: np.ndarray, freq_weights: np.ndarray) -> np.ndarray:
    x = np.ascontiguousarray(np.asarray(x, dtype=np.float32))
    freq_weights = np.asarray(freq_weights, dtype=np.float32)
    B = x.shape[0]
    assert x.shape == (32, 1, 512, 512) and freq_weights.shape == (64,)

    cstw, cstb, Wtile = _build_host_matrices_v2(freq_weights)
    uni = None
    if np.all(freq_weights == freq_weights[0]):
        uni = float(1.0 / (1.0 + np.exp(-float(freq_weights[0]))))
    if uni not in _NC_CACHE:
        _NC_CACHE[uni] = _build_bass_v2(cfg={"uniform_w": uni})
    nc = _NC_CACHE[uni]

    per = B // N_CORES
    in_maps = []
    for k in range(N_CORES):
        m = {
            "x": x[k * per : (k + 1) * per].reshape(per * 512, 512),
            "cstw": cstw,
            "cstb": cstb,
        }
        if uni is None:
            m["wt"] = Wtile
        in_maps.append(m)
    res = run_bass_kernel_spmd(nc, in_maps, list(range(N_CORES))).results
    # device layout: [(img,s), p, (v,l,q)] bf16 -> [B,64,64,64] f32
    # with ch = u*5+v (<40), hb = s*32 + t*2 + l, wb = q;
    # p = (u8,t16) in the c8 layout (else (t16,u8))
    raw = np.stack([res[k]["out"] for k in range(N_CORES)], axis=0)
    raw = raw.reshape(N_CORES * IMGS_PER_CORE, SLABS_PER_IMG, 16, 8, 5, 2, 64)
    ch40 = raw.transpose(0, 3, 4, 1, 2, 5, 6).reshape(B, 40, 64, 64)
    full = np.zeros((B, 64, 64, 64), dtype=np.float32)
    full[:, :40] = ch40.astype(np.float32)
    return full



# revision 38
# speedup vs baseline: 1.0068x; 1.0068x over previous
"""Trainium2 Bass kernel for the 8x8-block rfft2 magnitude ("DCT") layer.

Computes, for input x [32,1,512,512] f32 and freq_weights [64] f32:
  per 8x8 spatial block: |rfft2(block, norm='ortho')| -> 40 freq bins,
  scaled by sigmoid(freq_weights)[:40], zero-padded to 64 channels.
Output: [32, 64, 64, 64] f32.

Pure data parallel: 4 images per core on 8 cores. Per core, 8 slabs of
256 rows ([128, 1024] paired loads, one DMA each). The separable 8-pt
DFT runs as two matmul stages with a PSUM->SBUF bf16 copy between:

  stage 1 (per 128-col chunk q, per row-half h): data stationary
    (fp32r), W1 moving (fp32r, N=256; fp32r needs N>=256 for
    1 col/cycle): vertical DFT of every row-block, transposed so
    spatial columns (bjl,j) land on partitions.
  stage 2 (per chunk, per row-parity l): Z stationary (bf16 contiguous
    slices), [C|S] / [-S|C] bf16 moving (N=160): accumulating matmul
    pair gives Re/Im of the 2D DFT at psum partitions p=(t,u).

  Elementwise tail per slab: 2 Z-copies [128,1024] PSUM->SBUF bf16
    (DVE - the serial chain that sets the steady-state cadence,
    ~1.2us each; HW allows only ONE PSUM input per instruction and
    GPSIMD has no PSUM port, so the drain work can only go to
    DVE/ACT), 2 squares [128,640] PSUM->SBUF bf16 (ACT), 4 bf16 adds
    (GPSIMD/Pool), 1 sqrt [128,640] bf16 (ACT) with sigmoid(w) folded
    into the activation scale when freq_weights is uniform.

  Store: one contiguous [128, 640] bf16 tile per slab (p=(t,u),
    f=(v,l,q)) into a raw [8,128,640] bf16 output on the ACT HWDGE
    ring (st=scalar, ~-2us vs sharing the SP ring with loads); the
    host reorders/casts to [B,64,64,64] f32 and fills ch 40..63 with
    zeros. Halves store traffic vs f32 and replaces the old scattered
    512B-run store with 1280B-contiguous runs.

  Emission is software-pipelined (front k / mid k-4 / tail k-7). The
  repeat-loop timing path unrolls the body (unroll_rep) because each
  For_i staggered-reset boundary costs ~9us of pipeline drain; ur=16
  amortizes it to ~0.5us.

  HW-measured (repeat-loop slope, 8 cores concurrent, full problem per
  iteration): ~21.85us/core vs 33.1us for the session-start baseline
  (f32 store) and ~50.4us for v1. The plateau is real: DVE copy chain
  19.4us + ~0.3us/slab sem/handoff latency. Also tested and rejected:
  mid-before-front emission (mfirst - PE queue reorder, +0.8us),
  per-slab copy migration to ACT (queue coupling), DMA-assisted PSUM
  drain (fabric budget + no dma-cast), ur=32 (icache pressure). Steady-state floor is the 16-copy
  DVE chain (~19.4us) + sem slack; DMA is ~2.1us/slab and fully
  hidden. Failed experiments (kept as cfg options, off by default):
  c8=1 compact-Z via conjugate symmetry - the model loves it but HW
  runs multi-dim strided/short-run engine copies 3-4x slower than the
  1 el/cycle model, 74us total; per-slab copy-engine alternation
  (cps) similarly loses to queue head-of-line coupling.
"""

import math
import numpy as np
from contextlib import ExitStack

import ml_dtypes
import concourse.bacc as bacc
import concourse.mybir as mybir
from concourse import tile
from concourse.bass_utils import run_bass_kernel_spmd

F32 = mybir.dt.float32
F32R = mybir.dt.float32r
BF16 = mybir.dt.bfloat16

N_CORES = 8
IMGS_PER_CORE = 4
SLABS_PER_IMG = 2  # 512 rows / 256


def _build_host_matrices_v2(
    freq_weights: np.ndarray, c8: bool = False, mini: bool = False
):
    p = np.arange(128)
    bi_r, i_r = p // 8, p % 8
    # W1 [128, 256]: n = (l2, reim2, thl8, u8) -- or (l2, reim2, u8, thl8)
    # for the c8 layout (u-outer m-blocks keep the compact-copy runs
    # contiguous); bil = 2*thl + l; DFT /8
    n = np.arange(256)
    if mini:
        # mini-slab (128-row) W1: n = (reim2, t16, u8), block-row t
        reim_n = n // 128
        t_n = (n % 128) // 8
        u_n = n % 8
        ang = 2 * math.pi * np.outer(i_r, u_n) / 8.0
        W1 = np.where(reim_n[None, :] == 0, np.cos(ang), np.sin(ang)) / 8.0
        W1 *= bi_r[:, None] == t_n[None, :]
        return _finish_host_matrices(W1, freq_weights, c8)
    l_n = n // 128
    reim_n = (n % 128) // 64
    if c8:
        u_n = (n % 64) // 8
        th_n = n % 8
    else:
        th_n = (n % 64) // 8
        u_n = n % 8
    bil_n = 2 * th_n + l_n
    ang = 2 * math.pi * np.outer(i_r, u_n) / 8.0
    W1 = np.where(reim_n[None, :] == 0, np.cos(ang), np.sin(ang)) / 8.0
    W1 *= bi_r[:, None] == bil_n[None, :]
    return _finish_host_matrices(W1, freq_weights, c8)


def _finish_host_matrices(W1, freq_weights, c8):
    p = np.arange(128)
    # C2/S2 [128 p=(bjl,j), 80 m=(v,bjl2)]
    bjl_p, j_p = p // 8, p % 8
    m = np.arange(80)
    v_m, bjl2_m = m // 16, m % 16
    blk = bjl_p[:, None] == bjl2_m[None, :]
    ang2 = 2 * math.pi * v_m[None, :] * j_p[:, None] / 8.0
    C2 = np.cos(ang2) * blk
    S2 = np.sin(ang2) * blk
    CS = np.concatenate([C2, S2], axis=1)
    SNC = np.concatenate([-S2, C2], axis=1)
    cstw = W1.astype(np.float32)  # [128, 256] fp32r stage-1 stream
    cstb = np.concatenate([CS, SNC], axis=1)  # [128, 320]
    cstb = cstb.astype(np.float32).astype(ml_dtypes.bfloat16)

    # Wtile [128 p, 640 f=(v,l,q)] -> sigmoid(fw)[u*5+v]
    # p = (u8,t16) for c8 layout, else (t16,u8)
    w = 1.0 / (1.0 + np.exp(-freq_weights.astype(np.float64)))
    u_idx = np.arange(128) // 16 if c8 else np.arange(128) % 8
    v_idx = np.arange(640) // 128
    Wtile = w[u_idx[:, None] * 5 + v_idx[None, :]].astype(ml_dtypes.bfloat16)
    return cstw, cstb, Wtile


_NC_CACHE = {}


def _build_bass_v3(n_imgs: int = IMGS_PER_CORE, repeat: int = 1, cfg: dict = None):
    """Mini-slab variant: 16 slabs of 128 rows. Each mini needs only
    2 psz + 2 pso PSUM banks, so TWO minis are in flight (vs one 256-row
    slab owning all 8 banks) -- the psz recycle distance doubles and the
    per-copy sem/handoff slack overlaps the other mini's copy."""
    cfg = dict(cfg or {})
    uni_w = cfg.get("uniform_w")
    cp_eng = cfg.get("cp", "v")
    sq_eng = cfg.get("sq", "a")
    add_eng = cfg.get("add", "g")
    wm_eng = cfg.get("wm", "g")
    ld = None
    st = None
    ab = cfg.get("a", 8)
    zb = cfg.get("z", 10)
    sqb = cfg.get("sqb", 10)
    ssb = cfg.get("ss", 5)
    rtb = cfg.get("rt", 5)
    d_mid = cfg.get("dmid", 5)
    d_tail = cfg.get("dtail", 3)  # pairs lag, in minis

    MS = 4  # minis per image
    nc = bacc.Bacc("TRN2", target_bir_lowering=False)
    x = nc.dram_tensor("x", [n_imgs * 512, 512], F32R, kind="ExternalInput")
    cstw = nc.dram_tensor("cstw", [128, 256], F32R, kind="ExternalInput")
    cstb = nc.dram_tensor("cstb", [128, 320], BF16, kind="ExternalInput")
    if uni_w is None:
        wt = nc.dram_tensor("wt", [128, 640], BF16, kind="ExternalInput")
    # raw layout: [pair=(img,2), 128, 640]; pair = 2 minis
    out = nc.dram_tensor(
        "out", [n_imgs * 2, 128, 640], BF16, kind="ExternalOutput"
    )

    with tile.TileContext(nc) as tc, ExitStack() as ctx:
        consts = ctx.enter_context(tc.tile_pool(name="consts", bufs=1))
        a_pool = ctx.enter_context(tc.tile_pool(name="a", bufs=ab))
        z_pool = ctx.enter_context(tc.tile_pool(name="z", bufs=zb))
        sq_pool = ctx.enter_context(tc.tile_pool(name="sq", bufs=sqb))
        ss_pool = ctx.enter_context(tc.tile_pool(name="ss", bufs=ssb))
        rt_pool = ctx.enter_context(tc.tile_pool(name="rt", bufs=rtb))
        psz_pool = ctx.enter_context(
            tc.tile_pool(name="psz", bufs=2, space="PSUM")
        )
        pso_pool = ctx.enter_context(
            tc.tile_pool(name="pso", bufs=2, space="PSUM")
        )
        ld = getattr(nc, cfg.get("ld", "sync"))
        st = getattr(nc, cfg.get("st", "scalar"))

        def do_copy(eng, dst, srcv):
            if eng == "v":
                nc.vector.tensor_copy(dst, srcv)
            else:
                nc.scalar.copy(dst, srcv)

        cstw_t = consts.tile([128, 256], F32R, tag="cstw")
        cst_t = consts.tile([128, 320], BF16, tag="cstb")
        nc.sync.dma_start(cstw_t[:], cstw[:])
        nc.sync.dma_start(cst_t[:], cstb[:])
        w1_t = cstw_t[:]
        cs_t = cst_t[:, 0:160]
        snc_t = cst_t[:, 160:320]
        if uni_w is None:
            wt_t = consts.tile([128, 640], BF16, tag="wt")
            nc.sync.dma_start(wt_t[:], wt[:])

        warm = consts.tile([128, 8], F32, tag="warm")
        nc.gpsimd.memset(warm[:], 0.0)
        nc.scalar.square(warm[:], warm[:])
        nc.scalar.sqrt(warm[:], warm[:])

        minis = [
            (img, ms) for img in range(n_imgs) for ms in range(MS)
        ]
        unroll_rep = cfg.get("unroll_rep", 0)
        if unroll_rep:
            minis = minis * unroll_rep

        def emit_front(img, ms):
            a_t = a_pool.tile([128, 512], F32R)
            row0 = img * 512 + ms * 128
            ld.dma_start(a_t[:], x[row0 : row0 + 128, :])
            psz_t = psz_pool.tile([128, 1024], F32, tag="psz")
            for c in range(4):
                nc.tensor.matmul(
                    psz_t[:, 256 * c : 256 * c + 256],
                    a_t[:, 128 * c : 128 * c + 128],
                    w1_t, start=True, stop=True,
                )
            z_t = z_pool.tile([128, 1024], BF16)
            do_copy(cp_eng, z_t[:], psz_t[:])
            return z_t

        def emit_mid(z_t, mhalf, ss_pair):
            """stage 2 + square + add for one mini; results land in the
            mhalf half of the shared per-pair ssum super."""
            pso_t = pso_pool.tile([128, 1024], F32, tag="pso")
            offs = (0, 160, 512, 672)
            for c in range(4):
                dst = pso_t[:, offs[c] : offs[c] + 160]
                nc.tensor.matmul(
                    dst, z_t[:, 256 * c : 256 * c + 128],
                    cs_t, start=True, stop=False,
                )
                nc.tensor.matmul(
                    dst, z_t[:, 256 * c + 128 : 256 * c + 256],
                    snc_t, start=False, stop=True,
                )
            sq_t = sq_pool.tile([128, 640], BF16, tag="sq")
            po_v = pso_t[:].rearrange("p (b x) -> p b x", b=2, x=512)[
                :, :, 0:320
            ]
            sq_v = sq_t[:].rearrange("p (b x) -> p b x", b=2, x=320)
            if sq_eng == "a":
                nc.scalar.square(sq_v, po_v)
            else:
                nc.vector.tensor_mul(sq_v, po_v, po_v)
            # sq f = (c4, ReIm2, v5, b16); add re+im -> ss (c4, v5, b16)
            sq6 = sq_t[:].rearrange(
                "p (c r v b) -> p c r v b", c=4, r=2, v=5, b=16
            )
            ss_v = ss_pair[:, 320 * mhalf : 320 * mhalf + 320].rearrange(
                "p (c v b) -> p c v b", c=4, v=5, b=16
            )
            (nc.gpsimd.tensor_add if add_eng == "g"
             else nc.vector.tensor_add)(
                ss_v, sq6[:, :, 0], sq6[:, :, 1]
            )

        def emit_tail(img, pr, ss_pair):
            root_t = rt_pool.tile([128, 640], BF16, tag="root")
            if uni_w is not None:
                nc.scalar.activation(
                    root_t[:], ss_pair[:],
                    mybir.ActivationFunctionType.Sqrt,
                    0.0, float(uni_w) * float(uni_w),
                )
            else:
                rootf_t = rt_pool.tile([128, 640], F32, tag="rootf")
                nc.scalar.sqrt(rootf_t[:], ss_pair[:])
                (nc.gpsimd.tensor_mul if wm_eng == "g"
                 else nc.vector.tensor_mul)(
                    root_t[:], rootf_t[:], wt_t[:]
                )
            st.dma_start(out[img * 2 + pr], root_t[:])

        stag = cfg.get("stag", 1)
        rep_ctx = (
            tc.For_i(0, repeat, 1, staggered_reset=bool(stag))
            if repeat > 1 else None
        )
        if rep_ctx is not None:
            rep_ctx.__enter__()

        fronts, mids = [], []  # mids: (img, pair, ss_pair) per DONE pair
        ss_cur = None
        for si, (img, ms) in enumerate(minis):
            fronts.append((img, ms, emit_front(img, ms)))
            if len(fronts) > d_mid:
                fi, fs, fz = fronts.pop(0)
                if fs % 2 == 0:
                    ss_cur = ss_pool.tile([128, 640], BF16, tag="ssum")
                emit_mid(fz, fs % 2, ss_cur)
                if fs % 2 == 1:
                    mids.append((fi, fs // 2, ss_cur))
            if len(mids) > d_tail:
                mi, mp, mss = mids.pop(0)
                emit_tail(mi, mp, mss)
        while fronts:
            fi, fs, fz = fronts.pop(0)
            if fs % 2 == 0:
                ss_cur = ss_pool.tile([128, 640], BF16, tag="ssum")
            emit_mid(fz, fs % 2, ss_cur)
            if fs % 2 == 1:
                mids.append((fi, fs // 2, ss_cur))
            while len(mids) > d_tail:
                mi, mp, mss = mids.pop(0)
                emit_tail(mi, mp, mss)
        while mids:
            mi, mp, mss = mids.pop(0)
            emit_tail(mi, mp, mss)
        if rep_ctx is not None:
            rep_ctx.__exit__(None, None, None)
    nc.finalize()
    return nc


def _build_bass_v2(n_imgs: int = IMGS_PER_CORE, repeat: int = 1, cfg: dict = None):
    cfg = dict(cfg or {})
    if cfg.get("mini"):
        return _build_bass_v3(n_imgs, repeat, cfg)
    uni_w = cfg.get("uniform_w")
    cp_eng = cfg.get("cp", "vv")    # z-copy per super: v=DVE, a=ACT
    sq_eng = cfg.get("sq", "aa")    # square per po super: a=ACT, v=DVE
    add_eng = cfg.get("add", "gggg")  # add per po: g=GPSIMD(Pool), v=DVE
    wm_eng = cfg.get("wm", "g")
    ld_ring = cfg.get("ld", "sync")
    st_ring = cfg.get("st", "scalar")
    ab = cfg.get("a", 6)
    zb = cfg.get("z", 10)     # [128,1024] bf16 super tiles; 2 per slab
    sqb = cfg.get("sqb", 10)   # [128, 640]; 2 per slab
    ssb = cfg.get("ss", 7)
    rtb = cfg.get("rt", 7)
    pszb = cfg.get("psz", 2)  # 2-bank super-tiles
    psob = cfg.get("pso", 2)  # 2-bank super-tiles

    nc = bacc.Bacc("TRN2", target_bir_lowering=False)
    x = nc.dram_tensor("x", [n_imgs * 512, 512], F32R, kind="ExternalInput")
    cstw = nc.dram_tensor("cstw", [128, 256], F32R, kind="ExternalInput")
    cstb = nc.dram_tensor("cstb", [128, 320], BF16, kind="ExternalInput")
    if uni_w is None:
        wt = nc.dram_tensor("wt", [128, 640], BF16, kind="ExternalInput")
    # raw bf16 layout: one contiguous [128, 640] tile per slab
    # (p=(t,u), f=(v,l,q)); host reorders to [B,64,64,64] f32. Halves
    # store traffic vs f32 and stores in 1280B-contig runs per partition
    # instead of 512B scattered runs.
    out = nc.dram_tensor(
        "out", [n_imgs * SLABS_PER_IMG, 128, 640], BF16, kind="ExternalOutput"
    )

    ld = getattr(nc, ld_ring)
    st = getattr(nc, st_ring)

    with tile.TileContext(nc) as tc, ExitStack() as ctx:
        consts = ctx.enter_context(tc.tile_pool(name="consts", bufs=1))
        a_pool = ctx.enter_context(tc.tile_pool(name="a", bufs=ab))
        z_pool = ctx.enter_context(tc.tile_pool(name="z", bufs=zb))
        sq_pool = ctx.enter_context(tc.tile_pool(name="sq", bufs=sqb))
        ss_pool = ctx.enter_context(tc.tile_pool(name="ss", bufs=ssb))
        rt_pool = ctx.enter_context(tc.tile_pool(name="rt", bufs=rtb))
        psz_pool = ctx.enter_context(tc.tile_pool(name="psz", bufs=pszb, space="PSUM"))
        pso_pool = ctx.enter_context(tc.tile_pool(name="pso", bufs=psob, space="PSUM"))

        def do_copy(eng, dst, src):
            if eng == "v":
                nc.vector.tensor_copy(dst, src)
            elif eng == "g":
                nc.gpsimd.tensor_copy(dst, src)
            else:
                nc.scalar.copy(dst, src)

        def do_square(eng, dst, src):
            if eng == "a":
                nc.scalar.square(dst, src)
            elif eng == "g":
                nc.gpsimd.tensor_mul(dst, src, src)
            else:
                nc.vector.tensor_mul(dst, src, src)

        def do_add(eng, dst, a, b):
            (nc.gpsimd.tensor_add if eng == "g" else nc.vector.tensor_add)(
                dst, a, b
            )

        def do_dup(eng, dst, src):
            do_copy(eng, dst, src)

        def do_negdup(eng, dst, src):
            if eng == "g":
                nc.gpsimd.tensor_scalar_mul(dst, src, -1.0)
            elif eng == "a":
                nc.scalar.activation(
                    dst, src, mybir.ActivationFunctionType.Copy, 0.0, -1.0
                )
            else:
                nc.vector.tensor_scalar_mul(dst, src, -1.0)

        # pre-issue the first x-slab loads ahead of the const DMAs so the
        # load pipeline starts at t~0 (consts only gate stage-1 matmuls).
        # only valid for repeat==1 (inside a repeat loop slabs reload).
        npre = cfg.get("npre", 0) if repeat == 1 else 0
        pre_a = []
        for pi in range(npre):
            img, s = pi // SLABS_PER_IMG, pi % SLABS_PER_IMG
            a_t = a_pool.tile([128, 1024], F32R)
            row0 = img * 512 + s * 256
            src = x[row0 : row0 + 256, :].rearrange("(h p) c -> p h c", h=2)
            ld.dma_start(a_t[:].rearrange("p (h c) -> p h c", h=2), src)
            pre_a.append(a_t)

        cstw_t = consts.tile([128, 256], F32R, tag="cstw")
        cst_t = consts.tile([128, 320], BF16, tag="cstb")
        nc.sync.dma_start(cstw_t[:], cstw[:])  # W1 first
        nc.sync.dma_start(cst_t[:], cstb[:])
        w1_t = cstw_t[:]
        cs_t = cst_t[:, 0:160]
        snc_t = cst_t[:, 160:320]
        if uni_w is None:
            wt_t = consts.tile([128, 640], BF16, tag="wt")
            nc.sync.dma_start(wt_t[:], wt[:])

        # warm ACT tables (Square, Sqrt) at t=0
        warm = consts.tile([128, 8], F32, tag="warm")
        nc.gpsimd.memset(warm[:], 0.0)
        nc.scalar.square(warm[:], warm[:])
        nc.scalar.sqrt(warm[:], warm[:])
        # warm the PE p-state during the const/first-load DMA window:
        # junk matmuls keep PE continuously busy so the first real
        # stage-1 runs at full clock (cold PE is ~3.7x slower)
        npew = cfg.get("pew", 6)
        if npew:
            warm2 = consts.tile([128, 512], BF16, tag="warm2")
            nc.vector.memset(warm2[:], 0.0)
            pw_t = pso_pool.tile([128, 1024], F32, tag="po")
            for _ in range(npew):
                nc.tensor.matmul(
                    pw_t[:, 0:512], warm2[:, 0:128], warm2[:],
                    start=True, stop=True,
                )

        slabs = [(img, s) for img in range(n_imgs) for s in range(SLABS_PER_IMG)]

        psing = cfg.get("psing", 0)  # 1: psz/z per-chunk 1-bank tiles
        lda = cfg.get("lda")    # per-slab load-ring rotation, e.g. ["sync","scalar"]
        sta = cfg.get("sta")    # per-slab store-ring rotation
        c8 = cfg.get("c8", 0)   # compact-z: copy 10/16 stage-1 comps, mirror rest
        dre_eng = cfg.get("dre", "v")  # reR dup engine
        dim_eng = cfg.get("dim", "g")  # imR' negated-dup engine
        if psing:
            c8 = 0

        def emit_front(img, s, a_pre=None, cp=None):
            """load + stage 1 + Z copies; psz f = (r2,h2,thl8,u8,l2), t=8h+thl.
            psing: per-chunk [128,512] psz/z tiles (fine-grained rotation);
            else 2-bank super-tiles with one copy per chunk-pair."""
            cp = cp or cp_eng
            if a_pre is not None:
                a_t = a_pre
            else:
                a_t = a_pool.tile([128, 1024], F32R)
                row0 = img * 512 + s * 256
                src = x[row0 : row0 + 256, :].rearrange("(h p) c -> p h c", h=2)
                ldq = ld
                if lda:
                    ldq = getattr(nc, lda[(img * SLABS_PER_IMG + s) % len(lda)])
                if not nold:
                    ldq.dma_start(
                        a_t[:].rearrange("p (h c) -> p h c", h=2), src
                    )
            z_ts = []
            if psing:
                for q in range(4):
                    psz_t = psz_pool.tile([128, 512], F32, tag="psz")
                    psz5 = psz_t[:].rearrange(
                        "p (lr h m) -> p lr h m", lr=4, h=2, m=64
                    )
                    for h in range(2):
                        nc.tensor.matmul(
                            psz5[:, :, h, :],
                            a_t[:, 512 * h + 128 * q : 512 * h + 128 * q + 128],
                            w1_t,
                            start=True,
                            stop=True,
                        )
                    z_t = z_pool.tile([128, 512], BF16)
                    do_copy(cp[q], z_t[:], psz_t[:])
                    z_ts.append(z_t[:])
                return z_ts
            for g in range(2):
                psz_t = psz_pool.tile([128, 1024], F32, tag="psz")
                for c in range(2):
                    q = 2 * g + c
                    psz5 = psz_t[:, 512 * c : 512 * c + 512].rearrange(
                        "p (lr h m) -> p lr h m", lr=4, h=2, m=64
                    )
                    for h in range(2):
                        nc.tensor.matmul(
                            psz5[:, :, h, :],
                            a_t[:, 512 * h + 128 * q : 512 * h + 128 * q + 128],
                            w1_t,
                            start=True,
                            stop=True,
                        )
                z_t = z_pool.tile([128, 1024], BF16)
                if c8:
                    # compact copy: only u=0..4 of re/im (conjugate
                    # symmetry makes u=5..7 mirrors), transposing to
                    # u-outer within each (c,l,reim) 128-block:
                    #   z block = [re0..4 | reR(5..7)] / [im0..4 | imR']
                    src_v = psz_t[:].rearrange(
                        "p (m h u t) -> p m u h t", m=8, h=2, u=8, t=8
                    )[:, :, 0:5]
                    dst_v = z_t[:].rearrange(
                        "p (m u h t) -> p m u h t", m=8, u=8, h=2, t=8
                    )[:, :, 0:5]
                    do_copy(cp[g], dst_v, src_v)
                    # mirrors: re(8-u) = re(u); stored-im(8-u) = -im(u)
                    zv = z_t[:].rearrange(
                        "p (cl r u ht) -> p cl r u ht", cl=4, r=2, u=8, ht=16
                    )
                    do_dup(dre_eng, zv[:, :, 0, 5:8, :], zv[:, :, 0, 3:0:-1, :])
                    do_negdup(
                        dim_eng, zv[:, :, 1, 5:8, :], zv[:, :, 1, 3:0:-1, :]
                    )
                else:
                    do_copy(cp[g], z_t[:], psz_t[:])
                z_ts.append(z_t[:, 0:512])
                z_ts.append(z_t[:, 512:1024])
            return z_ts

        osup = cfg.get("osup", 1)  # 1: pso 2-bank supers; 0: 1-bank po tiles

        def emit_mid(z_ts, sq=None, ad=None):
            """stage 2 + squares + adds -> ssum tile. osup: pso super-tile g
            holds po(l=0) at 0:320, po(l=1) at 512:832 for chunk-pair g with
            one square per super; else per-po 1-bank tiles."""
            sq = sq or sq_eng
            ad = ad or add_eng
            ssum_t = ss_pool.tile([128, 640], BF16, tag="ssum")
            ss6 = ssum_t[:].rearrange(
                "p (v l P c b) -> p l P c v b", v=5, l=2, P=2, c=2, b=16
            )
            for pair in range(2):
                if osup:
                    po_t = pso_pool.tile([128, 1024], F32, tag="po")
                    po_parts = [
                        po_t[:, 0:320],
                        po_t[:, 512:832],
                    ]
                else:
                    po_parts = None
                sqs = []
                for l in range(2):
                    if osup:
                        dst_po = po_parts[l]
                    else:
                        po_s = pso_pool.tile([128, 320], F32, tag="po")
                        dst_po = po_s[:]
                    for ci, q in enumerate((2 * pair, 2 * pair + 1)):
                        zq = z_ts[q]
                        dst = dst_po[:, 160 * ci : 160 * ci + 160]
                        nc.tensor.matmul(
                            dst, zq[:, 256 * l : 256 * l + 128],
                            cs_t, start=True, stop=False,
                        )
                        nc.tensor.matmul(
                            dst, zq[:, 256 * l + 128 : 256 * l + 256],
                            snc_t, start=False, stop=True,
                        )
                    if not osup:
                        sq_t = sq_pool.tile([128, 320], BF16, tag="sq")
                        do_square(sq[2 * pair + l], sq_t[:], dst_po)
                        sqs.append(sq_t)
                if osup:
                    # one square over both l-halves of the super
                    sq_t = sq_pool.tile([128, 640], BF16, tag="sq")
                    po_v = po_t[:].rearrange(
                        "p (l m) -> p l m", l=2, m=512
                    )[:, :, 0:320]
                    sq_v = sq_t[:].rearrange("p (l m) -> p l m", l=2, m=320)
                    do_square(sq[pair], sq_v, po_v)
                    sq6 = sq_t[:].rearrange(
                        "p (li c h v b) -> p li h c v b", li=2, c=2, h=2, v=5, b=16
                    )
                    sq_ins = [(sq6[:, l, 0], sq6[:, l, 1]) for l in range(2)]
                else:
                    sq_ins = []
                    for l in range(2):
                        s5 = sqs[l][:].rearrange(
                            "p (c h v b) -> p h c v b", c=2, h=2, v=5, b=16
                        )
                        sq_ins.append((s5[:, 0], s5[:, 1]))
                for l in range(2):
                    do_add(
                        ad[2 * pair + l],
                        ss6[:, l, pair], sq_ins[l][0], sq_ins[l][1],
                    )
            return ssum_t

        def emit_tail(img, s, ssum_t, split=False):
            """sqrt(+weight) + store (bf16 raw tile). split: 2 half-ops
            (halves the last-slab drain latency; +1 ACT op bubble)"""
            root_t = rt_pool.tile([128, 640], BF16, tag="root")
            halves = ((0, 320), (320, 640)) if split else ((0, 640),)
            for lo, hi in halves:
                if uni_w is not None:
                    nc.scalar.activation(
                        root_t[:, lo:hi], ssum_t[:, lo:hi],
                        mybir.ActivationFunctionType.Sqrt,
                        0.0, float(uni_w) * float(uni_w),
                    )
                else:
                    rootf_t = rt_pool.tile([128, 640], F32, tag="rootf")
                    nc.scalar.sqrt(rootf_t[:, lo:hi], ssum_t[:, lo:hi])
                    (nc.gpsimd.tensor_mul if wm_eng == "g"
                     else nc.vector.tensor_mul)(
                        root_t[:, lo:hi], rootf_t[:, lo:hi], wt_t[:, lo:hi]
                    )
                stq = st
                if sta:
                    stq = getattr(nc, sta[(img * SLABS_PER_IMG + s) % len(sta)])
                if not nost:
                    stq.dma_start(
                        out[img * SLABS_PER_IMG + s, :, lo:hi], root_t[:, lo:hi]
                    )

        # software-pipelined emission: each engine's strict-FIFO queue gets
        # work ordered by data availability (front k+1 before mid k before
        # tail k-1), so no stage head-of-line-blocks the next slab's work.
        d_mid = cfg.get("dmid", 4)    # front-to-mid emission lag (slabs)
        d_tail = cfg.get("dtail", 3)  # mid-to-tail lag

        unroll_rep = cfg.get("unroll_rep", 0)
        if unroll_rep:
            slabs = slabs * unroll_rep
        stag = cfg.get("stag", 1)
        if repeat > 1 and cfg.get("pbar", 0):
            tc.prologue_barrier()
        rep_ctx = (
            tc.For_i(0, repeat, 1, staggered_reset=bool(stag))
            if repeat > 1
            else None
        )
        if rep_ctx is not None:
            rep_ctx.__enter__()
        # explicit staggered-reset stage boundaries at slab boundaries
        # (auto-split cuts mid-slab and skews the 4 stage mini-barriers)
        sbnd = cfg.get("sbnd", 0)
        bnd_after = (
            {len(slabs) // 4 - 1, len(slabs) // 2 - 1, 3 * len(slabs) // 4 - 1}
            if (rep_ctx is not None and stag and sbnd)
            else set()
        )
        nold = cfg.get("nold", 0)   # ablation: skip load DMAs
        nost = cfg.get("nost", 0)   # ablation: skip store DMAs
        nomid = cfg.get("nomid", 0)  # ablation: front stage only
        cps = cfg.get("cps")    # optional per-slab cp strings
        sqs = cfg.get("sqs")
        adds = cfg.get("adds")
        rsp = cfg.get("rsp", 0)  # split tail for the last rsp slabs
        nsl = len(slabs)
        tail_i = 0

        def pop_tail(mi, ms, mss):
            nonlocal tail_i
            emit_tail(mi, ms, mss, split=tail_i >= nsl - rsp)
            tail_i += 1

        mfirst = cfg.get("mfirst", 0)  # emit mid/tail before front each
        # round: ready mm2/sq work precedes mm1 that waits on PSUM
        # recycling, avoiding PE-queue head-of-line blocking
        fronts, mids = [], []
        for si, (img, s) in enumerate(slabs):
            a_pre = pre_a[si] if si < npre else None
            cp_s = cps[si % len(cps)] if cps else None
            if mfirst:
                if len(fronts) >= d_mid:
                    fi, fs, fz = fronts.pop(0)
                    fsi = si - d_mid
                    if not nomid:
                        mids.append((fi, fs, emit_mid(
                            fz,
                            sq=sqs[fsi % len(sqs)] if sqs else None,
                            ad=adds[fsi % len(adds)] if adds else None,
                        )))
                if len(mids) > d_tail:
                    mi, ms, mss = mids.pop(0)
                    pop_tail(mi, ms, mss)
                fronts.append(
                    (img, s, emit_front(img, s, a_pre=a_pre, cp=cp_s))
                )
                if si in bnd_after:
                    tc.stage_boundary()
                continue
            fronts.append(
                (img, s, emit_front(img, s, a_pre=a_pre, cp=cp_s))
            )
            if len(fronts) > d_mid:
                fi, fs, fz = fronts.pop(0)
                fsi = si - d_mid
                if not nomid:
                    mids.append((fi, fs, emit_mid(
                        fz,
                        sq=sqs[fsi % len(sqs)] if sqs else None,
                        ad=adds[fsi % len(adds)] if adds else None,
                    )))
            if si in bnd_after:
                tc.stage_boundary()
            if len(mids) > d_tail:
                mi, ms, mss = mids.pop(0)
                pop_tail(mi, ms, mss)
        nxt = len(slabs) - len(fronts)
        while fronts:
            fi, fs, fz = fronts.pop(0)
            if not nomid:
                mids.append((fi, fs, emit_mid(
                    fz,
                    sq=sqs[nxt % len(sqs)] if sqs else None,
                    ad=adds[nxt % len(adds)] if adds else None,
                )))
            nxt += 1
            while len(mids) > d_tail:
                mi, ms, mss = mids.pop(0)
                pop_tail(mi, ms, mss)
        while mids:
            mi, ms, mss = mids.pop(0)
            pop_tail(mi, ms, mss)
        if rep_ctx is not None:
            rep_ctx.__exit__(None, None, None)
    nc.finalize()
    return nc


def kernel(x: np.ndarray, freq_weights: np.ndarray) -> np.ndarray:
    x = np.ascontiguousarray(np.asarray(x, dtype=np.float32))
    freq_weights = np.asarray(freq_weights, dtype=np.float32)
    B = x.shape[0]
    assert x.shape == (32, 1, 512, 512) and freq_weights.shape == (64,)

    cstw, cstb, Wtile = _build_host_matrices_v2(freq_weights)
    uni = None
    if np.all(freq_weights == freq_weights[0]):
        uni = float(1.0 / (1.0 + np.exp(-float(freq_weights[0]))))
    if uni not in _NC_CACHE:
        _NC_CACHE[uni] = _build_bass_v2(cfg={"uniform_w": uni})
    nc = _NC_CACHE[uni]

    per = B // N_CORES
    in_maps = []
    for k in range(N_CORES):
        m = {
            "x": x[k * per : (k + 1) * per].reshape(per * 512, 512),
            "cstw": cstw,
            "cstb": cstb,
        }
        if uni is None:
            m["wt"] = Wtile
        in_maps.append(m)
    res = run_bass_kernel_spmd(nc, in_maps, list(range(N_CORES))).results
    # device layout: [(img,s), p, (v,l,q)] bf16 -> [B,64,64,64] f32
    # with ch = u*5+v (<40), hb = s*32 + t*2 + l, wb = q;
    # p = (u8,t16) in the c8 layout (else (t16,u8))
    raw = np.stack([res[k]["out"] for k in range(N_CORES)], axis=0)
    raw = raw.reshape(N_CORES * IMGS_PER_CORE, SLABS_PER_IMG, 16, 8, 5, 2, 64)
    ch40 = raw.transpose(0, 3, 4, 1, 2, 5, 6).reshape(B, 40, 64, 64)
    full = np.zeros((B, 64, 64, 64), dtype=np.float32)
    full[:, :40] = ch40.astype(np.float32)
    return full

